# revision 14
# baseline (speedup 1.0000x reference)
"""Trainium2 Bass kernel for nn_DynamicsShaper: time-varying RBJ lowpass biquad
driven by per-segment-averaged logits.

Sharding: batch row r -> NeuronCore r (8 rows, 8 cores, fully independent).

v2: f16 inputs (host casts; segment ids < 512 are exact in f16) halve input
DMA; gates and the forcing path run in f16 on DVE; the ACT chain is
reordered (sigmoids -> exps -> sin -> recip -> cos -> gain sigmoid) to
minimize table reloads; Pool (GpSimd) takes tensor_tensor-only work (gain
channel correction + one output-correction group).  The order-2 blocked
scan (50 chunks x 25 samples, double-steps, Hillis-Steele map composition)
is unchanged from v1.
"""

import sys

sys.path.insert(0, "/opt/trn_rl_repo")

import numpy as np

import concourse.bass as bass
import concourse.bacc as bacc
import concourse.mybir as mybir
import concourse.tile as tile
from concourse import masks

P = 128          # SBUF partitions
W = 1250         # samples per partition (T = P*W)
C = 50           # chunks per partition
L = W // C       # chunk length (25); 12 pair-steps + 1 single step
T = P * W
B = 8
SR = 16000.0
GAIN_MIN, GAIN_MAX = 0.1, 2.0
LOG_MIN_W = float(np.log(2.0 * np.pi * 20.0 / SR))
LOG_MAX_W = float(np.log(np.pi))
LOG_MIN_Q, LOG_MAX_Q = float(np.log(0.0707)), float(np.log(2.0))

fp = mybir.dt.float32
f16 = mybir.dt.float16
OP = mybir.AluOpType
AF = mybir.ActivationFunctionType

# knobs (flip by measurement)
F16_SCANS = True      # f16 gate/data inputs to the segmented scans
POOL_D0CORR = False   # gain-channel correction on Pool via 2 tensor_tensor
POOL_CORRGRP = True   # last output-correction group on Pool


def _act_recip(nc, out, in_, bias=0.0, scale=1.0):
    """ACT-table reciprocal 1/(scale*x + bias); refine with Newton after."""
    eng = nc.scalar
    inputs = [
        eng.lower_ap(in_),
        mybir.ImmediateValue(dtype=mybir.dt.float32, value=float(bias)),
        mybir.ImmediateValue(dtype=mybir.dt.float32, value=float(scale)),
        mybir.ImmediateValue(dtype=mybir.dt.float32, value=0.0),
    ]
    return eng.add_instruction(
        mybir.InstActivation(
            name=nc.get_next_instruction_name(),
            func=AF.Reciprocal,
            ins=inputs,
            outs=[eng.lower_ap(out)],
        )
    )


def build_program():
    nc = bacc.Bacc("TRN2", target_bir_lowering=False, debug=False, num_devices=B)
    d_seg = nc.dram_tensor("seg", [P, W], f16, kind="ExternalInput").ap()
    d_c1 = nc.dram_tensor("c1", [P, W], f16, kind="ExternalInput").ap()
    d_c2 = nc.dram_tensor("c2", [P, W], f16, kind="ExternalInput").ap()
    d_c0n = nc.dram_tensor("c0n", [P, 2 * W], f16, kind="ExternalInput").ap()
    d_bnd = nc.dram_tensor("bnd", [P, 2], f16, kind="ExternalInput").ap()
    d_y = nc.dram_tensor("y", [P, W], fp, kind="ExternalOutput").ap()
    with tile.TileContext(nc) as tc:
        _body(nc, tc, d_seg, d_c1, d_c2, d_c0n, d_bnd, d_y)
    nc.compile()
    return nc


def _body(nc, tc, d_seg, d_c1, d_c2, d_c0n, d_bnd, d_y):
    from contextlib import ExitStack
    ctx = ExitStack()
    pool = ctx.enter_context(tc.tile_pool(name="main", bufs=1))
    psum = ctx.enter_context(tc.tile_pool(name="ps", bufs=1, space="PSUM"))

    V = nc.vector
    G = nc.gpsimd
    A = nc.scalar

    # ---------- loads (seg first: it gates the first compute) ----------
    seg = pool.tile([P, W], f16)
    c1 = pool.tile([P, W], f16)
    c2 = pool.tile([P, W], f16)
    c0n = pool.tile([P, 2 * W], f16)
    cmp = pool.tile([P, W + 1], f16)
    nc.sync.dma_start(seg[:], d_seg)
    nc.sync.dma_start(cmp[:, 0:1], d_bnd[:, 0:1])
    nc.sync.dma_start(cmp[:, W:W + 1], d_bnd[:, 1:2])
    nc.sync.dma_start(c1[:], d_c1)
    nc.sync.dma_start(c2[:], d_c2)
    nc.sync.dma_start(c0n[:], d_c0n)
    c0 = c0n[:, 0:W]
    noise16 = c0n[:, W:2 * W]

    # ---------- constants: identity + shift matrices ----------
    ident = pool.tile([P, P], fp)
    masks.make_identity(nc, ident[:])
    ident8 = pool.tile([8, 8], fp)
    masks.make_identity(nc, ident8[:])

    zmat = pool.tile([P, P], fp)
    G.memset(zmat[:], 0.0)

    def shift_mat(base):
        m = pool.tile([P, P], fp, name=f"shift_{base}")
        G.affine_select(out=m[:], in_=zmat[:], compare_op=OP.not_equal, fill=1.0,
                        base=base, pattern=[[-1, P]], channel_multiplier=1)
        return m

    sh_up = {s: shift_mat(s) for s in (1, 2, 4, 8, 16, 32, 64)}  # out[p] = in[p-s]

    # identity-affine pads for HS rounds: rows < s get identity map
    # map layout per 6 cols: (d1, p1, q1, d2, p2, q2); identity: p1=1, q2=1
    idpad = {}
    for s in (1, 2, 4, 8, 16, 32, 64):
        t = pool.tile([P, 6], fp, name=f"idpad_{s}")
        V.memset(t[:], 0.0)
        V.memset(t[0:s, 1:2], 1.0)
        V.memset(t[0:s, 5:6], 1.0)
        idpad[s] = t

    # small scalar-bias constants (set up while input DMA is in flight)
    one1 = pool.tile([P, 1], fp)
    V.memset(one1[:], 1.0)
    one1h = pool.tile([P, 1], f16)
    V.memset(one1h[:], 1.0)
    bias_w = pool.tile([P, 1], fp)
    V.memset(bias_w[:], LOG_MIN_W)
    bias_q = pool.tile([P, 1], fp)
    V.memset(bias_q[:], -LOG_MIN_Q - float(np.log(2.0)))  # folds alpha's 0.5
    bias_hp = pool.tile([P, 1], fp)
    V.memset(bias_hp[:], float(np.pi / 2))

    # ---------- gates (f16; only DVE reads them) ----------
    V.tensor_tensor(cmp[:, 1:W], seg[:, 1:], seg[:, :W - 1], OP.is_equal)
    g = cmp[:, 0:W]
    e = cmp[:, 1:W + 1]
    sameseg = pool.tile([P, 1], f16)
    V.tensor_tensor(sameseg[:], seg[:, 0:1], seg[:, W - 1:W], OP.is_equal)
    gPc = pool.tile([P, 1], f16)
    V.tensor_tensor(gPc[:], sameseg[:], cmp[:, 0:1], OP.mult)
    gRc = pool.tile([P, 1], f16)
    V.tensor_tensor(gRc[:], sameseg[:], cmp[:, W:W + 1], OP.mult)

    # fp32 copies of the boundary gates (scalar operands must be fp32)
    bnd0f = pool.tile([P, 1], fp)
    V.tensor_copy(bnd0f[:], cmp[:, 0:1])
    bndWf = pool.tile([P, 1], fp)
    V.tensor_copy(bndWf[:], cmp[:, W:W + 1])

    ie = pool.tile([P, W], f16)
    A.activation(ie[:], e, AF.Identity, scale=-1.0, bias=one1[:])  # 1-e

    # ---------- forward scans + gates, all DVE, w/q channels first -------
    l0 = pool.tile([P, W], fp)
    d0 = [pool.tile([P, W], fp, name=f"d0_{c}") for c in range(3)]
    Gp = pool.tile([P, W], f16)
    Erev = pool.tile([P, W], f16)

    V.tensor_tensor_scan(l0[:], g, one1h[:].to_broadcast([P, W]), 0.0,
                         OP.mult, OP.add)
    V.tensor_tensor(Gp[:], seg[:], seg[:, 0:1].to_broadcast([P, W]),
                    OP.is_equal)
    V.tensor_tensor_scan(d0[1][:], g, c1[:], 0.0, OP.mult, OP.add)
    V.tensor_tensor_scan(d0[2][:], g, c2[:], 0.0, OP.mult, OP.add)
    V.tensor_tensor(Erev[:], seg[:], seg[:, W - 1:W].to_broadcast([P, W]),
                    OP.is_equal)

    # ---------- cross-partition chain helpers ----------
    def chain_fwd(tails, tag):
        """Exclusive chain over partitions for forward scans."""
        n = len(tails)
        sA = pool.tile([P, 2 * n], fp, name=f"s_{tag}")
        V.tensor_copy(sA[:, 0:n], gPc[:].to_broadcast([P, n]))
        for i, tl in enumerate(tails):
            V.tensor_copy(sA[:, n + i:n + i + 1], tl)
        pg = psum.tile([4, P], fp, tag="pg_ch")
        pd = psum.tile([4, P], fp, tag="pd_ch")
        nc.tensor.transpose(pg[0:n, :], sA[:, 0:n], ident[:])
        nc.tensor.transpose(pd[0:n, :], sA[:, n:2 * n], ident[:])
        tg = pool.tile([n, P], fp, name=f"tg_{tag}")
        td = pool.tile([n, P], fp, name=f"td_{tag}")
        V.tensor_copy(tg[:], pg[0:n, :])
        V.tensor_copy(td[:], pd[0:n, :])
        chv = pool.tile([n, P], fp, name=f"chv_{tag}")
        V.tensor_tensor_scan(chv[:], tg[:], td[:], 0.0, OP.mult, OP.add)
        shv = pool.tile([n, P], fp, name=f"shv_{tag}")
        V.memset(shv[:, 0:1], 0.0)
        V.tensor_copy(shv[:, 1:P], chv[:, 0:P - 1])
        pc = psum.tile([P, 4], fp, tag="pc_ch")
        nc.tensor.matmul(pc[:, 0:n], shv[:], ident8[0:n, 0:n])
        dv = pool.tile([P, n], fp, name=f"dIn_{tag}")
        V.tensor_scalar_mul(dv[:], pc[:, 0:n], bnd0f[:])
        return dv

    def chain_rev_ph1(heads, tag):
        n = len(heads)
        sA = pool.tile([P, 2 * n], fp, name=f"s_{tag}")
        V.tensor_copy(sA[:, 0:n], gRc[:].to_broadcast([P, n]))
        for i, hd in enumerate(heads):
            V.tensor_copy(sA[:, n + i:n + i + 1], hd)
        pg = psum.tile([4, P], fp, tag="pg_rv")
        pd = psum.tile([4, P], fp, tag="pd_rv")
        nc.tensor.transpose(pg[0:n, :], sA[:, 0:n], ident[:])
        nc.tensor.transpose(pd[0:n, :], sA[:, n:2 * n], ident[:])
        return (n, tag, pg, pd)

    def chain_rev_ph2(st):
        n, tag, pg, pd = st
        tg = pool.tile([n, P], fp, name=f"tg_{tag}")
        td = pool.tile([n, P], fp, name=f"td_{tag}")
        V.tensor_copy(tg[:], pg[0:n, :])
        V.tensor_copy(td[:], pd[0:n, :])
        chv = pool.tile([n, P], fp, name=f"chv_{tag}")
        V.tensor_tensor_scan(chv[:, ::-1], tg[:, ::-1], td[:, ::-1],
                             0.0, OP.mult, OP.add)
        shv = pool.tile([n, P], fp, name=f"shv_{tag}")
        V.memset(shv[:, P - 1:P], 0.0)
        V.tensor_copy(shv[:, 0:P - 1], chv[:, 1:P])
        pc = psum.tile([P, 4], fp, tag="pc_rv")
        nc.tensor.matmul(pc[:, 0:n], shv[:], ident8[0:n, 0:n])
        return (n, tag, pc)

    def chain_rev_ph3(st):
        n, tag, pc = st
        dv = pool.tile([P, n], fp, name=f"mIn_{tag}")
        V.tensor_scalar_mul(dv[:], pc[:, 0:n], bndWf[:])
        return dv

    # ---------- corrections + run means ----------
    l = pool.tile([P, W], fp)
    rl = pool.tile([P, W], f16)
    h = pool.tile([P, W], f16)
    d1t = seg  # seg dead after Gp/Erev
    dat = [pool.tile([P, W], f16, name=f"dat_{c}") for c in range(3)]
    m0 = [pool.tile([P, W], fp, name=f"m0_{c}") for c in range(3)]

    # last-run AND continues-into-next-partition gate (for sigmoid fixups)
    ErevC = pool.tile([P, W], f16)
    V.tensor_scalar_mul(ErevC[:], Erev[:], bndWf[:])
    ieC = pool.tile([P, W], f16)   # 1 - ErevC
    V.tensor_scalar(ieC[:], ErevC[:], -1.0, 1.0, OP.mult, OP.add)

    with tc.high_priority():
        dInL = chain_fwd([l0[:, W - 1:W]], "fl")
        dIn1 = chain_fwd([d0[1][:, W - 1:W]], "f1")
        V.scalar_tensor_tensor(l[:], Gp[:], dInL[:, 0:1], l0[:],
                               OP.mult, OP.add)
        _act_recip(nc, rl[:], l[:])
        V.scalar_tensor_tensor(d1t[:], Gp[:], dIn1[:, 0:1], d0[1][:],
                               OP.mult, OP.add)
        dIn2 = chain_fwd([d0[2][:, W - 1:W]], "f2")
        V.scalar_tensor_tensor(d0[2][:], Gp[:], dIn2[:, 0:1], d0[2][:],
                               OP.mult, OP.add)
        V.tensor_tensor(h[:], ie[:], rl[:], OP.mult)
        V.tensor_tensor(dat[1][:], d1t[:], h[:], OP.mult)
        V.tensor_tensor_scan(m0[1][:, ::-1], e[:, ::-1], dat[1][:, ::-1],
                             0.0, OP.mult, OP.add)
        r1 = chain_rev_ph1([m0[1][:, 0:1]], "r1")
        V.tensor_tensor(dat[2][:], d0[2][:], h[:], OP.mult)
        r1b = chain_rev_ph2(r1)
        mIn1 = chain_rev_ph3(r1b)
        V.tensor_tensor_scan(m0[2][:, ::-1], e[:, ::-1], dat[2][:, ::-1],
                             0.0, OP.mult, OP.add)
        r2 = chain_rev_ph1([m0[2][:, 0:1]], "r2")
        r2b = chain_rev_ph2(r2)
        mIn2 = chain_rev_ph3(r2b)

    # ---------- ACT chain + sigmoid-select fixups ----------
    # The rev scans leave exactly 0 in a partition's continuing last run, so
    # sigmoid() of the uncorrected scan is wrong only there; fix with the
    # per-partition sigmoid(mIn) via the ErevC gate.  This takes the chain
    # round-trip off the sigmoid critical path.
    sg1e = c1      # c1 dead after d1 scan
    sg2e = c2      # c2 dead after d2 scan
    sg0e = pool.tile([P, W], f16)
    sg1h = pool.tile([P, W], f16)
    sg2h = pool.tile([P, W], f16)
    sg0h = pool.tile([P, W], f16)
    qinvh = pool.tile([P, W], f16)   # 1/(2q)
    sinwh = pool.tile([P, W], f16)
    alphah = pool.tile([P, W], f16)  # alpha = sin(w)/(2q)
    r0a = pool.tile([P, W], f16)     # ~1/(1+alpha), table seed
    w = d0[1]                        # dead after d1 correction
    cosw = l0                        # dead after l correction
    sgmIn1 = pool.tile([P, 1], fp)
    sgmIn2 = pool.tile([P, 1], fp)
    sgmIn0 = pool.tile([P, 1], fp)

    def sg_fix(out, sge, sgmIn, tscr=None, uscr=None):
        V.tensor_tensor(out[:], sge[:], ieC[:], OP.mult)
        V.scalar_tensor_tensor(out[:], ErevC[:], sgmIn[:, 0:1], out[:],
                               OP.mult, OP.add)

    with tc.high_priority():
        A.activation(sg1e[:], m0[1][:], AF.Sigmoid)
        A.activation(sgmIn1[:], mIn1[:, 0:1], AF.Sigmoid)
        A.activation(sg2e[:], m0[2][:], AF.Sigmoid)
        A.activation(sgmIn2[:], mIn2[:, 0:1], AF.Sigmoid)
        sg_fix(sg1h, sg1e, sgmIn1)
        sg_fix(sg2h, sg2e, sgmIn2)
        A.activation(qinvh[:], sg2h[:], AF.Exp, bias=bias_q[:],
                     scale=-(LOG_MAX_Q - LOG_MIN_Q))
        A.activation(w[:], sg1h[:], AF.Exp, bias=bias_w[:],
                     scale=(LOG_MAX_W - LOG_MIN_W))
        A.activation(sinwh[:], w[:], AF.Sin)
        V.tensor_tensor(alphah[:], sinwh[:], qinvh[:], OP.mult)
        _act_recip(nc, r0a[:], alphah[:], bias=1.0)

    # gain channel: scans scheduled after w/q (fills the ACT-chain window)
    V.tensor_tensor_scan(d0[0][:], g, c0, 0.0, OP.mult, OP.add)
    dIn0 = chain_fwd([d0[0][:, W - 1:W]], "f0")
    if POOL_D0CORR:
        gtmp = l  # fp32, dead after rl; reused again as yfinA later
        G.tensor_tensor(gtmp[:], Gp[:], dIn0[:, 0:1].to_broadcast([P, W]),
                        OP.mult)
        G.tensor_tensor(d0[0][:], gtmp[:], d0[0][:], OP.add)
    else:
        V.scalar_tensor_tensor(d0[0][:], Gp[:], dIn0[:, 0:1], d0[0][:],
                               OP.mult, OP.add)
    V.tensor_tensor(dat[0][:], d0[0][:], h[:], OP.mult)
    V.tensor_tensor_scan(m0[0][:, ::-1], e[:, ::-1], dat[0][:, ::-1],
                         0.0, OP.mult, OP.add)
    rb = chain_rev_ph1([m0[0][:, 0:1]], "rb")
    rbb = chain_rev_ph2(rb)
    mInB = chain_rev_ph3(rbb)

    A.activation(cosw[:], w[:], AF.Sin, bias=bias_hp[:], scale=-1.0)
    A.activation(sg0e[:], m0[0][:], AF.Sigmoid)
    A.activation(sgmIn0[:], mInB[:, 0:1], AF.Sigmoid)
    sg_fix(sg0h, sg0e, sgmIn0)

    # ---------- Newton + biquad coefficients (DVE) ----------
    nsc2 = d0[2]
    inva0 = pool.tile([P, W], fp)
    with tc.high_priority():
        V.scalar_tensor_tensor(nsc2[:], alphah[:], 1.0, r0a[:],
                               OP.add, OP.mult)              # (1+alpha)*r0
        nsc3 = m0[1]  # fp32, dead after m1 correction
        V.tensor_scalar(nsc3[:], nsc2[:], -1.0, 2.0, OP.mult, OP.add)
        V.tensor_tensor(inva0[:], nsc3[:], r0a[:], OP.mult)
    b0pre = pool.tile([P, W], f16)
    V.tensor_scalar(b0pre[:], cosw[:], -0.5, 0.5, OP.mult, OP.add)
    na1 = d0[1]   # w dead after sin/cos
    na2 = d0[0]   # dead after dat0
    with tc.high_priority():
        V.scalar_tensor_tensor(na1[:], cosw[:], 2.0, inva0[:],
                               OP.mult, OP.mult)
        V.scalar_tensor_tensor(na2[:], alphah[:], 1.0, inva0[:],
                               OP.subtract, OP.mult)
    b016 = pool.tile([P, W], f16)
    V.tensor_tensor(b016[:], b0pre[:], inva0[:], OP.mult)

    # ---------- gain / x / FIR path (fp16) ----------
    gain16 = sg2h  # dead after qinvh
    V.tensor_scalar(gain16[:], sg0h[:], GAIN_MAX - GAIN_MIN, GAIN_MIN,
                    OP.mult, OP.add)
    x = pool.tile([P, W], f16)
    V.tensor_tensor(x[:], noise16, gain16[:], OP.mult)
    xt32 = pool.tile([P, 2], fp)
    V.tensor_copy(xt32[:], x[:, W - 2:W])
    ps_x = psum.tile([P, 2], fp, tag="ps_small")
    nc.tensor.matmul(ps_x[:], sh_up[1][:], xt32[:])
    xb = pool.tile([P, 2], fp)   # (x[p-1, W-2], x[p-1, W-1]); row0 = 0
    V.tensor_copy(xb[:], ps_x[:])
    s_f = pool.tile([P, W], f16)
    f_t = pool.tile([P, W], f16)
    V.scalar_tensor_tensor(s_f[:, 2:], x[:, 1:W - 1], 2.0, x[:, 2:],
                           OP.mult, OP.add)
    V.tensor_tensor(f_t[:, 2:], s_f[:, 2:], x[:, :W - 2], OP.add)
    V.scalar_tensor_tensor(s_f[:, 0:1], xb[:, 1:2], 2.0, x[:, 0:1],
                           OP.mult, OP.add)
    V.tensor_tensor(f_t[:, 0:1], s_f[:, 0:1], xb[:, 0:1], OP.add)
    V.scalar_tensor_tensor(s_f[:, 1:2], x[:, 0:1], 2.0, x[:, 1:2],
                           OP.mult, OP.add)
    V.tensor_tensor(f_t[:, 1:2], s_f[:, 1:2], xb[:, 1:2], OP.add)
    fsc = pool.tile([P, W], f16)
    V.tensor_tensor(fsc[:], f_t[:], b016[:], OP.mult)
    f = fsc

    # ---------- double-step composite coefficients ----------
    Lh = L // 2
    na13 = na1.rearrange("p (c n) -> p c n", c=C)
    na23 = na2.rearrange("p (c n) -> p c n", c=C)
    f3 = f.rearrange("p (c n) -> p c n", c=C)
    n1e = na13[:, :, 0:2 * Lh:2]
    n1o = na13[:, :, 1:2 * Lh:2]
    n2e = na23[:, :, 0:2 * Lh:2]
    n2o = na23[:, :, 1:2 * Lh:2]
    Amt = pool.tile([P, C * Lh], fp)
    Amt3 = Amt.rearrange("p (c m) -> p c m", c=C)
    V.tensor_tensor(Amt3[:], n1o, n1e, OP.mult)
    Amf = pool.tile([P, C * Lh], fp)
    Amf3 = Amf.rearrange("p (c m) -> p c m", c=C)
    V.tensor_tensor(Amf3[:], Amt3[:], n2o, OP.add)
    Bmf = pool.tile([P, C * Lh], fp)
    Bmf3 = Bmf.rearrange("p (c m) -> p c m", c=C)
    V.tensor_tensor(Bmf3[:], n1o, n2e, OP.mult)
    fD = pool.tile([P, C * Lh * 2], fp)
    fD4 = fD.rearrange("p (c m k) -> p c m k", c=C, m=Lh, k=2)
    coefD = pool.tile([P, C * Lh * 12], fp)
    cD4 = coefD.rearrange("p (c m k) -> p c m k", c=C, m=Lh, k=12)
    for mlo, mhi in ((0, 2), (2, 6), (6, Lh)):
        fe = f3[:, :, 2 * mlo:2 * mhi:2]
        fo = f3[:, :, 2 * mlo + 1:2 * mhi:2]
        n1o_s = na13[:, :, 2 * mlo + 1:2 * mhi:2]
        V.tensor_tensor(fD4[:, :, mlo:mhi, 1:2], n1o_s.unsqueeze(3),
                        fe.unsqueeze(3), OP.mult)
        V.tensor_tensor(fD4[:, :, mlo:mhi, 1:2], fD4[:, :, mlo:mhi, 1:2],
                        fo.unsqueeze(3), OP.add)
        V.tensor_copy(fD4[:, :, mlo:mhi, 0:1], fe.unsqueeze(3))
        nm = mhi - mlo
        A.activation(cD4[:, :, mlo:mhi, 0:3],
                     n2e[:, :, mlo:mhi].unsqueeze(3).to_broadcast(
                         [P, C, nm, 3]), AF.Copy)
        A.activation(cD4[:, :, mlo:mhi, 3:6],
                     n1e[:, :, mlo:mhi].unsqueeze(3).to_broadcast(
                         [P, C, nm, 3]), AF.Copy)
        A.activation(cD4[:, :, mlo:mhi, 6:9],
                     Bmf3[:, :, mlo:mhi].unsqueeze(3).to_broadcast(
                         [P, C, nm, 3]), AF.Copy)
        A.activation(cD4[:, :, mlo:mhi, 9:12],
                     Amf3[:, :, mlo:mhi].unsqueeze(3).to_broadcast(
                         [P, C, nm, 3]), AF.Copy)

    # ---------- within-chunk recursions (y_zs, p, q interleaved) ----------
    CD = C
    ypqA = pool.tile([P, CD * (L + 2) * 3], fp)
    ypqA3 = ypqA.rearrange("p (c m) -> p c m", c=CD)
    V.memset(ypqA3[:, :, 0:6], 0.0)
    V.memset(ypqA3[:, :, 2:3], 1.0)   # q_{-2} = 1
    V.memset(ypqA3[:, :, 4:5], 1.0)   # p_{-1} = 1
    uA = pool.tile([P, CD * 12], fp)
    uA4 = uA.rearrange("p (c s k) -> p c s k", c=CD, s=2, k=6)
    parts = ((V, ypqA3, uA4, 0, CD),)
    for m in range(Lh):
        n = 2 * m
        for eng, y3t, u4t, lo, hi in parts:
            cn = hi - lo
            prevs = y3t[:, :, 3 * n:3 * n + 6].unsqueeze(2).to_broadcast(
                [P, cn, 2, 6])
            coefv = cD4[:, lo:hi, m, :].rearrange("p c (s k) -> p c s k",
                                                  s=2, k=6)
            eng.tensor_tensor(u4t[:], prevs, coefv, OP.mult)
            eng.tensor_tensor(
                y3t[:, :, 3 * n + 6:3 * n + 12].rearrange(
                    "p c (s k) -> p c s k", s=2, k=3),
                u4t[:, :, :, 0:3], u4t[:, :, :, 3:6], OP.add)
            eng.tensor_tensor(y3t[:, :, 3 * n + 6:3 * n + 10:3],
                              y3t[:, :, 3 * n + 6:3 * n + 10:3],
                              fD4[:, lo:hi, m, :], OP.add)
    if L % 2 == 1:
        n = L - 1
        pv2 = ypqA3[:, :, 3 * n:3 * n + 3]
        pv1 = ypqA3[:, :, 3 * n + 3:3 * n + 6]
        nw = ypqA3[:, :, 3 * n + 6:3 * n + 9]
        uL0 = uA.rearrange("p (c k) -> p c k", c=CD)[:, :, 0:3]
        uL1 = uA.rearrange("p (c k) -> p c k", c=CD)[:, :, 3:6]
        V.tensor_tensor(uL0[:], pv2,
                        na23[:, :, n:n + 1].to_broadcast([P, C, 3]), OP.mult)
        V.tensor_tensor(uL1[:], pv1,
                        na13[:, :, n:n + 1].to_broadcast([P, C, 3]), OP.mult)
        V.tensor_tensor(nw[:], uL0[:], uL1[:], OP.add)
        V.tensor_tensor(ypqA3[:, :, 3 * n + 6:3 * n + 7],
                        ypqA3[:, :, 3 * n + 6:3 * n + 7],
                        f3[:, :, n:n + 1], OP.add)

    # ---------- chunk-map prefix composition (log rounds along chunks) ----
    base = 3 * L
    mpa = pool.tile([P, C * 6], fp)
    mpb = pool.tile([P, C * 6], fp)
    ut = pool.tile([P, C * 12], fp)
    vt = pool.tile([P, C * 6], fp)
    mpa3 = mpa.rearrange("p (c k) -> p c k", c=C)
    src = ypqA3[:, :, base:base + 6].rearrange(
        "p c (r k) -> p c r k", r=2, k=3)[:, :, ::-1, :]
    V.tensor_copy(mpa3[:].rearrange("p c (r k) -> p c r k", r=2, k=3), src)
    cur, new = mpa, mpb
    s = 1
    while s < C:
        act = C - s
        c3 = cur.rearrange("p (c k) -> p c k", c=C)
        n3 = new.rearrange("p (c k) -> p c k", c=C)
        v3 = vt.rearrange("p (c k) -> p c k", c=C)
        u3 = ut.rearrange("p (c m) -> p c m", c=C)
        arows = c3[:, 0:act, :].rearrange("p c (k j) -> p c k j", k=2, j=3)
        for r in range(2):
            u4v = u3[:, 0:act, 6 * r:6 * r + 6].rearrange(
                "p c (k j) -> p c k j", k=2, j=3)
            bco = c3[:, s:C, 3 * r + 1:3 * r + 3].unsqueeze(3).to_broadcast(
                [P, act, 2, 3])
            V.tensor_tensor(u4v, bco, arows, OP.mult)
            V.tensor_tensor(v3[:, 0:act, 3 * r:3 * r + 3],
                            u3[:, 0:act, 6 * r:6 * r + 3],
                            u3[:, 0:act, 6 * r + 3:6 * r + 6], OP.add)
        V.tensor_tensor(n3[:, s:C, 0:4:3], v3[:, 0:act, 0:4:3],
                        c3[:, s:C, 0:4:3], OP.add)
        V.tensor_copy(
            n3[:, s:C, :].rearrange("p c (r k) -> p c r k", r=2, k=3)[
                :, :, :, 1:3],
            v3[:, 0:act, :].rearrange("p c (r k) -> p c r k", r=2, k=3)[
                :, :, :, 1:3])
        V.tensor_copy(n3[:, 0:s, :], c3[:, 0:s, :])
        cur, new = new, cur
        s *= 2
    cur3 = cur.rearrange("p (c k) -> p c k", c=C)
    Mcur = pool.tile([P, 6], fp)
    V.tensor_copy(Mcur[:], cur3[:, C - 1, :])

    # ---------- Hillis-Steele scan of affine maps over partitions ----
    Mnew = pool.tile([P, 6], fp)
    ash = pool.tile([P, 6], fp)
    v6 = pool.tile([P, 6], fp)
    u1t = pool.tile([P, 12], fp)
    ps_m = psum.tile([P, 6], fp)
    cur_m, new_m = Mcur, Mnew
    for s in (1, 2, 4, 8, 16, 32, 64):
        nc.tensor.matmul(ps_m[:], sh_up[s][:], cur_m[:])
        V.tensor_tensor(ash[:], ps_m[:], idpad[s][:], OP.add)
        bd = cur_m[:, 0:4:3].unsqueeze(2)                    # [P, 2, 1]
        a4 = ash.rearrange("p (t k) -> p t k", t=2).unsqueeze(1).to_broadcast(
            [P, 2, 2, 3])
        b4 = cur_m.rearrange("p (r k) -> p r k", r=2)[:, :, 1:3].unsqueeze(
            3).to_broadcast([P, 2, 2, 3])
        u1 = u1t.rearrange("p (r t k) -> p r t k", r=2, t=2)
        v = v6.rearrange("p (r k) -> p r k", r=2)
        nw = new_m.rearrange("p (r k) -> p r k", r=2)
        V.tensor_tensor(u1[:], a4, b4, OP.mult)
        V.tensor_tensor(v[:], u1[:, :, 0, :], u1[:, :, 1, :], OP.add)
        V.tensor_tensor(nw[:, :, 0:1], v[:, :, 0:1], bd, OP.add)
        V.tensor_copy(nw[:, :, 1:3], v[:, :, 1:3])
        cur_m, new_m = new_m, cur_m
    ps_d = psum.tile([P, 2], fp, tag="ps_small")
    nc.tensor.matmul(ps_d[:], sh_up[1][:], cur_m[:, 0:4:3])
    ab0 = pool.tile([P, 2], fp)   # (alpha0, beta0)
    V.tensor_copy(ab0[:], ps_d[:])

    # ---------- per-chunk incoming states ----------
    alc = pool.tile([P, C], fp)
    bec = pool.tile([P, C], fp)
    tq = pool.tile([P, C - 1], fp)
    V.tensor_copy(alc[:, 0:1], ab0[:, 0:1])
    V.tensor_copy(bec[:, 0:1], ab0[:, 1:2])
    ex3 = cur3[:, 0:C - 1, :]

    def excol(col):
        return ex3[:, :, col:col + 1].rearrange("p c k -> p (c k)")

    for dst, r in ((alc, 0), (bec, 1)):
        V.scalar_tensor_tensor(tq[:], excol(3 * r + 1), ab0[:, 0:1],
                               excol(3 * r), OP.mult, OP.add)
        V.scalar_tensor_tensor(dst[:, 1:C], excol(3 * r + 2), ab0[:, 1:2],
                               tq[:], OP.mult, OP.add)

    # ---------- correction pass: y = y_zs + p*alpha_c + q*beta_c ----------
    yfinA = l      # dead after rl
    t1A = inva0    # dead after na/b016
    t2A = m0[0]    # dead after m0 correction
    Cq = (C + 3) // 4
    bnds = [0, Cq, 2 * Cq, 3 * Cq, C]
    engs = [V, V, V, G if POOL_CORRGRP else V]
    for i in range(4):
        eng, lo, hi = engs[i], bnds[i], bnds[i + 1]
        cn = hi - lo
        y3t = ypqA3[:, lo:hi, :]
        pv = y3t[:, :, 7:6 + 3 * L:3]
        qv = y3t[:, :, 8:6 + 3 * L:3]
        yzs = y3t[:, :, 6:4 + 3 * L:3]
        alcv = alc[:, lo:hi].unsqueeze(2).to_broadcast([P, cn, L])
        becv = bec[:, lo:hi].unsqueeze(2).to_broadcast([P, cn, L])
        y3o = yfinA.rearrange("p (c n) -> p c n", c=C)[:, lo:hi, :]
        t13 = t1A.rearrange("p (c n) -> p c n", c=C)[:, lo:hi, :]
        t23 = t2A.rearrange("p (c n) -> p c n", c=C)[:, lo:hi, :]
        eng.tensor_tensor(t13[:], pv, alcv, OP.mult)
        eng.tensor_tensor(t23[:], qv, becv, OP.mult)
        eng.tensor_tensor(y3o[:], t13[:], yzs, OP.add)
        eng.tensor_tensor(y3o[:], y3o[:], t23[:], OP.add)
        if i % 2 == 0:
            nc.sync.dma_start(d_y[:, lo * L:hi * L], yfinA[:, lo * L:hi * L])
        else:
            A.dma_start(d_y[:, lo * L:hi * L], yfinA[:, lo * L:hi * L])


_NC_CACHE = None


def _get_nc():
    global _NC_CACHE
    if _NC_CACHE is None:
        _NC_CACHE = build_program()
    return _NC_CACHE


def make_in_maps(noise, seg, lg):
    maps = []
    for r in range(len(noise)):
        s2 = seg[r].reshape(P, W)
        bnd = np.zeros((P, 2), np.float16)
        bnd[1:, 0] = (s2[1:, 0] == s2[:-1, W - 1])
        bnd[:-1, 1] = (s2[1:, 0] == s2[:-1, W - 1])
        c0n = np.concatenate(
            [lg[r, :, 0].reshape(P, W), noise[r].reshape(P, W)],
            axis=1).astype(np.float16)
        maps.append({
            "seg": s2.astype(np.float16),
            "c1": lg[r, :, 1].reshape(P, W).astype(np.float16),
            "c2": lg[r, :, 2].reshape(P, W).astype(np.float16),
            "c0n": c0n,
            "bnd": bnd,
        })
    return maps


def kernel(noise_bursts, segment_ids, logits):
    from concourse.bass_utils import run_bass_kernel_spmd

    noise = np.asarray(noise_bursts, dtype=np.float32)
    seg = np.asarray(segment_ids).astype(np.int32)
    lg = np.asarray(logits, dtype=np.float32)
    assert noise.shape == (B, T) and seg.shape == (B, T) and lg.shape == (B, T, 3)

    nc = _get_nc()
    in_maps = make_in_maps(noise, seg, lg)
    res = run_bass_kernel_spmd(nc, in_maps, list(range(B)))
    out = np.stack([res.results[r]["y"].reshape(T) for r in range(B)])
    return out.astype(np.float32)
